# revision 16
# baseline (speedup 1.0000x reference)
"""Trainium2 Bass kernel for nn_MixvMFGrad (mixture-of-vMF log-density gradient).

Math (per row s of the batch, d=512, K=64 components):
    dots  = s @ mus^T                        [K]
    t_k   = delta_k + kappa_k * dots_k       (delta = coef - max coef, host fp64)
    e     = exp(t)                           (unnormalized weights)
    g     = e @ mus                          [d]
    q     = g . s
    out   = (g - q s) / ||g||

Device computes o = g - q s (unnormalized) and q; the norm is recovered on
the host via Pythagoras: since ||s|| = 1, ||o||^2 = ||g||^2 - q^2, so
r = 1/sqrt(||o||^2 + q^2) and out = o * r. This removes the Ge matmul, the
e*Ge product, and the whole on-device rsqrt chain (ACT Rsqrt is banned and
exp/rsqrt live in different ACT table sets).

Layout: everything transposed ([d, rows] / [K, rows]), with s pre-transposed
and fp16-packed on the host so the device does ZERO transposes. Per 512-row
supertile the engine budget is 9 PE matmuls (4 dots + 1 q-reduce-broadcast +
4 gT), 3 ACT ops (exp, A->fp16, negq->fp16), 3 Pool ops (u = e*A16,
tmp = sT*negq16 in two halves; SBUF-only, Pool has no PSUM port), and 2 DVE
adds (o = tmp + gT, the only PSUM-sourced elementwise). I/O is fp16 both
ways (51 MB/core total), sized against the ~150us/core DMA roofline.

The q-reduce lands directly in broadcast form: negq_bc = redq^T @ u where
redq's 128 identical columns are -1/kappa, so every output partition holds
-q[r] and the tangent update needs no cross-partition broadcast.

Precision (numpy-emulated): rel err ~4e-3 vs fp64 truth (gate 2e-2). fp16
value ranges are safe: |A|<=25, e<=~250 (bf16), |u|<=~6e3, |o|<=~40.
"""

import os
from contextlib import ExitStack

import numpy as np

import concourse.bass as bass
import concourse.tile as tile
from concourse import bacc
from concourse import bass_isa
from concourse import mybir
from concourse.bass_utils import run_bass_kernel_spmd

N_CORES = 8
BS = 200000
D = 512
K = 64
ROWS_PER_CORE = BS // N_CORES   # 25000
ST_ROWS = 512                   # rows per supertile
PAD_ROWS = 25088                # 49 supertiles of 512
N_ST = PAD_ROWS // ST_ROWS
F32 = mybir.dt.float32
F16 = mybir.dt.float16
BF16 = mybir.dt.bfloat16

LAST_RESULT = None  # test.py reads exec_time_ns off this


def build_nc(rows=PAD_ROWS):
    assert rows % ST_ROWS == 0
    n_st = rows // ST_ROWS
    nc = bacc.Bacc("TRN2", target_bir_lowering=False)

    # packed layouts: x_d[st, p, c*512 + r] = x[row = st*512 + r, dim = c*128 + p]
    sT_d = nc.dram_tensor("sT", [n_st, 128, 2048], F16, kind="ExternalInput")
    o_d = nc.dram_tensor("o", [n_st, 128, 2048], F16, kind="ExternalOutput")
    nq_d = nc.dram_tensor("nq", [n_st, ST_ROWS], F16, kind="ExternalOutput")
    muskT_d = nc.dram_tensor("muskT", [128, 4, K], F16, kind="ExternalInput")
    delta_d = nc.dram_tensor("delta", [K, 1], F32, kind="ExternalInput")
    musr_d = nc.dram_tensor("musr", [K, D], BF16, kind="ExternalInput")
    nkh_d = nc.dram_tensor("nkh", [K, 1], F32, kind="ExternalInput")

    AF = mybir.ActivationFunctionType

    sT_v = sT_d[:].rearrange("t p (c r) -> t p c r", r=ST_ROWS)
    o_v = o_d[:].rearrange("t p (c r) -> t p c r", r=ST_ROWS)
    nq_v = nq_d[:]

    with tile.TileContext(nc) as tc, ExitStack() as ctx:
        consts = ctx.enter_context(tc.tile_pool(name="consts", bufs=1))
        in_pool = ctx.enter_context(tc.tile_pool(name="in_pool", bufs=3))
        out_pool = ctx.enter_context(tc.tile_pool(name="out_pool", bufs=3))
        e_pool = ctx.enter_context(tc.tile_pool(name="e_pool", bufs=2))
        u_pool = ctx.enter_context(tc.tile_pool(name="u_pool", bufs=2))
        q_pool = ctx.enter_context(tc.tile_pool(name="q_pool", bufs=2))
        ps_A = ctx.enter_context(tc.tile_pool(name="ps_A", bufs=2, space="PSUM"))
        ps_G = ctx.enter_context(tc.tile_pool(name="ps_G", bufs=3, space="PSUM"))

        muskT_sb = consts.tile([128, 4, K], F16)
        nc.sync.dma_start(out=muskT_sb, in_=muskT_d[:])
        delta_sb = consts.tile([K, 1], F32)
        nc.sync.dma_start(out=delta_sb, in_=delta_d[:])
        musr_sb = consts.tile([K, D], BF16)
        nc.sync.dma_start(out=musr_sb, in_=musr_d[:])
        nkh_sb = consts.tile([K, 1], F32)
        nc.sync.dma_start(out=nkh_sb, in_=nkh_d[:])

        for st in range(n_st):
            sT_t = in_pool.tile([128, 4, ST_ROWS], F16, tag="sT")
            nc.sync.dma_start(out=sT_t, in_=sT_v[st])
            o_t = out_pool.tile([128, 4, ST_ROWS], F16, tag="o")

            # A = (kappa*dots)^T [K, rows], fp32 PSUM
            A = ps_A.tile([K, ST_ROWS], F32, tag="A")
            for c in range(4):
                nc.tensor.matmul(
                    A, muskT_sb[:, c, :], sT_t[:, c, :],
                    start=(c == 0), stop=(c == 3),
                )

            e_t = e_pool.tile([K, ST_ROWS], BF16, tag="e")
            nc.scalar.activation(e_t, A, AF.Exp, bias=delta_sb)
            # A16 = A * (-1/(2 kappa)): the q-reduction weights folded into
            # the PSUM drain's per-partition scale
            A16 = e_pool.tile([K, ST_ROWS], F16, tag="A16")
            nc.scalar.mul(A16, A, nkh_sb)

            # u = e * A16 written twice (Pool engine: SBUF-only operands);
            # the 128-channel all-reduce then gives -q = 2 * sum(u)/2 on
            # every partition with no PE matmul and no ACT drain
            u2 = u_pool.tile([128, ST_ROWS], F16, tag="u")
            nc.gpsimd.tensor_mul(u2[0:K, :], e_t, A16)
            nc.gpsimd.tensor_mul(u2[K:2 * K, :], e_t, A16)

            # gT matmuls first: they only need e_t, so the in-order PE queue
            # is not stalled waiting for u (Pool) before useful work
            gts = []
            for h in range(2):
                gt = ps_G.tile([128, 2, ST_ROWS], F32, tag="g")
                for c2 in range(2):
                    c = 2 * h + c2
                    nc.tensor.matmul(
                        gt[:, c2, :], musr_sb[:, 128 * c:128 * (c + 1)], e_t,
                        start=True, stop=True,
                    )
                gts.append(gt)

            # -q on all 128 partitions via gpsimd cross-partition all-reduce
            nq16 = q_pool.tile([128, ST_ROWS], F16, tag="nq16")
            nc.gpsimd.partition_all_reduce(
                nq16, u2, channels=128, reduce_op=bass_isa.ReduceOp.add)
            nc.sync.dma_start(out=nq_v[st:st + 1], in_=nq16[0:1, :])

            # tmp = sT * (-q): DVE 16-bit 2x mode, one merged op per half
            # (nq16 free-broadcast across the two chunks via stride-0 dim);
            # then o = tmp + gT (the only PSUM-sourced elementwise)
            nq_b = nq16[:].rearrange("p (o r) -> p o r", o=1).broadcast_to(
                [128, 2, ST_ROWS])
            for h in range(2):
                nc.vector.tensor_mul(
                    o_t[:, 2 * h:2 * h + 2, :], sT_t[:, 2 * h:2 * h + 2, :],
                    nq_b)
                nc.vector.tensor_add(
                    o_t[:, 2 * h:2 * h + 2, :], o_t[:, 2 * h:2 * h + 2, :],
                    gts[h])

            nc.sync.dma_start(out=o_v[st], in_=o_t)

    nc.finalize()
    return nc


def host_prep(alphas, mus, kappas):
    """Host-side fp64 precompute of the tiny per-component constants."""
    a = np.asarray(alphas, np.float64)
    m = np.asarray(mus, np.float64)
    k = np.asarray(kappas, np.float64)
    d = m.shape[1]
    nu = 0.5 * d - 1.0
    z = k / nu
    sq = np.sqrt(1.0 + z * z)
    eta = sq + np.log(z) - np.log1p(sq)
    t = 1.0 / sq
    u1 = (3.0 * t - 5.0 * t ** 3) / 24.0
    u2 = (81.0 * t ** 2 - 462.0 * t ** 4 + 385.0 * t ** 6) / 1152.0
    log_iv = (nu * eta - 0.5 * np.log(2.0 * np.pi * nu)
              - 0.25 * np.log1p(z * z) + np.log1p(u1 / nu + u2 / (nu * nu)))
    logC = d * (-0.5 * np.log(2.0 * np.pi)) + nu * np.log(k) - log_iv
    coef = np.log(a) + np.log(k) + logC
    delta = (coef - coef.max()).astype(np.float32).reshape(K, 1)

    musk = k[:, None] * m                      # kappa_k * mus_k
    # muskT[p, c, j] = musk[j, 128c + p]
    muskT = np.ascontiguousarray(
        musk.reshape(K, 4, 128).transpose(2, 1, 0)).astype(np.float16)
    musr = np.asarray(mus, np.float64).astype(mybir.dt.np(BF16))
    nkh = (-0.5 / k)[:, None].astype(np.float32)
    return dict(muskT=muskT, delta=delta, musr=musr, nkh=nkh)


def pack_shard(shard16):
    """[PAD_ROWS, 512] fp16 -> [N_ST, 128, 2048] packed transposed."""
    v = shard16.reshape(N_ST, ST_ROWS, 4, 128).transpose(0, 3, 2, 1)
    return np.ascontiguousarray(v).reshape(N_ST, 128, 4 * ST_ROWS)


_NC_CACHE = {}


def kernel(s, alphas, mus, kappas):
    global LAST_RESULT
    s = np.asarray(s, np.float32)
    consts = host_prep(alphas, mus, kappas)

    if PAD_ROWS not in _NC_CACHE:
        _NC_CACHE[PAD_ROWS] = build_nc(PAD_ROWS)
    nc = _NC_CACHE[PAD_ROWS]

    in_maps = []
    for c in range(N_CORES):
        shard = s[c * ROWS_PER_CORE:(c + 1) * ROWS_PER_CORE]
        pad = PAD_ROWS - shard.shape[0]
        if pad:
            shard = np.concatenate([shard, shard[:pad]], axis=0)
        in_maps.append({"sT": pack_shard(shard.astype(np.float16)), **consts})

    res = run_bass_kernel_spmd(
        nc, in_maps, list(range(N_CORES)),
        trace=bool(os.environ.get("MIXVMF_TRACE")),
    )
    LAST_RESULT = res

    outs = []
    for c in range(N_CORES):
        o = np.asarray(res.results[c]["o"])
        nq = np.asarray(res.results[c]["nq"], np.float32).reshape(PAD_ROWS)
        out = np.ascontiguousarray(
            o.view(np.float16).reshape(N_ST, 128, 4, ST_ROWS)
            .transpose(0, 3, 2, 1)).reshape(PAD_ROWS, D).astype(np.float32)
        q = -nq
        no2 = np.einsum("ij,ij->i", out, out)
        r = 1.0 / np.sqrt(no2 + q * q)
        out *= r[:, None]
        outs.append(out[:ROWS_PER_CORE])
    return np.concatenate(outs, axis=0)


# revision 18
# speedup vs baseline: 3.2817x; 3.2817x over previous
"""Trainium2 Bass kernel for nn_MixvMFGrad (mixture-of-vMF log-density gradient).

Math (per row s of the batch, d=512, K=64 components):
    dots  = s @ mus^T                        [K]
    t_k   = delta_k + kappa_k * dots_k       (delta = coef - max coef, host fp64)
    e     = exp(t)                           (unnormalized weights)
    g     = e @ mus                          [d]
    q     = g . s
    out   = (g - q s) / ||g||

Device computes o = g - q s (unnormalized) and q; the norm is recovered on
the host via Pythagoras: since ||s|| = 1, ||o||^2 = ||g||^2 - q^2, so
r = 1/sqrt(||o||^2 + q^2) and out = o * r. This removes the Ge matmul, the
e*Ge product, and the whole on-device rsqrt chain (ACT Rsqrt is banned and
exp/rsqrt live in different ACT table sets).

Layout: everything transposed ([d, rows] / [K, rows]), with s pre-transposed
and fp16-packed on the host so the device does ZERO transposes. I/O is fp16
both ways (51 MB/core total), sized against the ~150us/core DMA roofline.

The loop is SOFTWARE-PIPELINED one stage deep: iteration i issues
dots(i), negq(i-1), gT(i-1) on PE — every operand (e, u of stage i-1) was
produced during iteration i-1, so the nine N=512 matmuls run back-to-back
with no semaphore stalls. An unbroken PE stream matters twice: it removes
pipeline bubbles, and sustained execution is what lets the Tensor engine
ramp out of the mid pstate (measured 630ns/matmul at 1.2GHz effective vs
~390ns at full clock).

The q-reduce lands directly in broadcast form: negq_bc = redq^T @ u where
redq's 128 identical columns are 1 (the -1/kappa weights ride in A16's
per-partition scale), so every output partition holds -q[r] and the tangent
update needs no cross-partition broadcast. Engine split per supertile:
PE 9 matmuls; ACT 3 ops (nq16 drain, A16 drain, exp); Pool 1 op (u = e*A16;
SBUF-only, Pool has no PSUM port); DVE 2 muls (16-bit 2x mode) + 4 adds
(the only PSUM-sourced elementwise). PSUM: A 2 banks + negq 1 + gT 5 = 8.

Precision (numpy-emulated): rel err ~4e-3 vs fp64 truth (gate 2e-2),
measured 1.8e-3 on HW. fp16 ranges are safe: |A|<=25, e<=~250 (bf16),
|u|<=~1.5e3, |o|<=~40.
"""

import os
from contextlib import ExitStack

import numpy as np

import concourse.bass as bass
import concourse.tile as tile
from concourse import bacc
from concourse import mybir
from concourse.bass_utils import run_bass_kernel_spmd

N_CORES = 8
BS = 200000
D = 512
K = 64
ROWS_PER_CORE = BS // N_CORES   # 25000
ST_ROWS = 512                   # rows per supertile
PAD_ROWS = 25088                # 49 supertiles of 512
N_ST = PAD_ROWS // ST_ROWS
F32 = mybir.dt.float32
F16 = mybir.dt.float16
BF16 = mybir.dt.bfloat16

LAST_RESULT = None  # test.py reads exec_time_ns off this


def build_nc(rows=PAD_ROWS):
    assert rows % ST_ROWS == 0
    n_st = rows // ST_ROWS
    nc = bacc.Bacc("TRN2", target_bir_lowering=False)

    # packed layouts: x_d[st, p, c*R + r] = x[row = st*R + r, dim = c*128 + p]
    sT_d = nc.dram_tensor("sT", [n_st, 128, 4 * ST_ROWS], F16,
                          kind="ExternalInput")
    o_d = nc.dram_tensor("o", [n_st, 128, 4 * ST_ROWS], F16,
                         kind="ExternalOutput")
    nq_d = nc.dram_tensor("nq", [n_st, ST_ROWS], F16, kind="ExternalOutput")
    muskT_d = nc.dram_tensor("muskT", [128, 4, K], F16, kind="ExternalInput")
    delta_d = nc.dram_tensor("delta", [K, 1], F32, kind="ExternalInput")
    musr_d = nc.dram_tensor("musr", [K, D], BF16, kind="ExternalInput")
    nkh_d = nc.dram_tensor("nkh", [K, 1], F32, kind="ExternalInput")
    redq_d = nc.dram_tensor("redq", [K, 128], F16, kind="ExternalInput")

    AF = mybir.ActivationFunctionType

    sT_v = sT_d[:].rearrange("t p (c r) -> t p c r", r=ST_ROWS)
    o_v = o_d[:].rearrange("t p (c r) -> t p c r", r=ST_ROWS)
    nq_v = nq_d[:]

    with tile.TileContext(nc) as tc, ExitStack() as ctx:
        consts = ctx.enter_context(tc.tile_pool(name="consts", bufs=1))
        in_pool = ctx.enter_context(tc.tile_pool(name="in_pool", bufs=3))
        out_pool = ctx.enter_context(tc.tile_pool(name="out_pool", bufs=3))
        e_pool = ctx.enter_context(tc.tile_pool(name="e_pool", bufs=2))
        u_pool = ctx.enter_context(tc.tile_pool(name="u_pool", bufs=2))
        q_pool = ctx.enter_context(tc.tile_pool(name="q_pool", bufs=2))
        # PSUM budget (8 banks): A 1bank x 2bufs, negq 1 x 1, gT 1 x 5
        ps_A = ctx.enter_context(tc.tile_pool(name="ps_A", bufs=2, space="PSUM"))
        ps_Q = ctx.enter_context(tc.tile_pool(name="ps_Q", bufs=1, space="PSUM"))
        ps_G = ctx.enter_context(tc.tile_pool(name="ps_G", bufs=5, space="PSUM"))

        muskT_sb = consts.tile([128, 4, K], F16)
        nc.sync.dma_start(out=muskT_sb, in_=muskT_d[:])
        delta_sb = consts.tile([K, 1], F32)
        nc.sync.dma_start(out=delta_sb, in_=delta_d[:])
        musr_sb = consts.tile([K, D], BF16)
        nc.sync.dma_start(out=musr_sb, in_=musr_d[:])
        nkh_sb = consts.tile([K, 1], F32)
        nc.sync.dma_start(out=nkh_sb, in_=nkh_d[:])
        redq_sb = consts.tile([K, 128], F16)
        nc.sync.dma_start(out=redq_sb, in_=redq_d[:])

        prev = None  # state of stage i-1: dict(sT, o, e, u)
        for i in range(n_st + 1):
            cur = None
            if i < n_st:
                sT_t = in_pool.tile([128, 4, ST_ROWS], F16, tag="sT")
                nc.sync.dma_start(out=sT_t, in_=sT_v[i])
                o_t = out_pool.tile([128, 4, ST_ROWS], F16, tag="o")

                # A = (kappa*dots)^T [K, rows], fp32 PSUM
                A = ps_A.tile([K, ST_ROWS], F32, tag="A")
                for c in range(4):
                    nc.tensor.matmul(
                        A, muskT_sb[:, c, :], sT_t[:, c, :],
                        start=(c == 0), stop=(c == 3),
                    )
                cur = dict(sT=sT_t, o=o_t)

            if prev is not None:
                # -q(i-1) on all 128 partitions: redq cols are all ones,
                # u already carries the -1/kappa weights
                negq = ps_Q.tile([128, ST_ROWS], F32, tag="q")
                nc.tensor.matmul(negq, redq_sb, prev["u"], start=True,
                                 stop=True)
                nq16 = q_pool.tile([128, ST_ROWS], F16, tag="nq16")
                nc.scalar.copy(nq16, negq)
                nc.sync.dma_start(out=nq_v[i - 1:i], in_=nq16[0:1, :])

                # gT(i-1) per d-chunk
                gts = []
                for c in range(4):
                    gt = ps_G.tile([128, ST_ROWS], F32, tag="g")
                    nc.tensor.matmul(
                        gt, musr_sb[:, 128 * c:128 * (c + 1)], prev["e"],
                        start=True, stop=True)
                    gts.append(gt)

                # tmp = sT * (-q) (DVE 16-bit 2x, merged pairs, stride-0
                # broadcast of nq16), then o = tmp + gT per chunk
                po, ps = prev["o"], prev["sT"]
                nq_b = nq16[:].rearrange("p (o r) -> p o r", o=1).broadcast_to(
                    [128, 2, ST_ROWS])
                nc.vector.tensor_mul(po[:, 0:2, :], ps[:, 0:2, :], nq_b)
                nc.vector.tensor_mul(po[:, 2:4, :], ps[:, 2:4, :], nq_b)
                for c in range(4):
                    nc.vector.tensor_add(po[:, c, :], po[:, c, :], gts[c])
                nc.sync.dma_start(out=o_v[i - 1], in_=po)

            if cur is not None:
                # ACT drains for stage i (after stage i-1's nq16 in the ACT
                # queue): A16 = A * (-1/kappa), e = exp(A + delta)
                A16 = e_pool.tile([K, ST_ROWS], F16, tag="A16")
                nc.scalar.mul(A16, A, nkh_sb)
                e_t = e_pool.tile([K, ST_ROWS], BF16, tag="e")
                nc.scalar.activation(e_t, A, AF.Exp, bias=delta_sb)
                # u = e * A16 (Pool engine: SBUF-only operands)
                u_t = u_pool.tile([K, ST_ROWS], F16, tag="u")
                nc.gpsimd.tensor_mul(u_t, e_t, A16)
                cur["e"] = e_t
                cur["u"] = u_t

            prev = cur

    nc.finalize()
    return nc


def host_prep(alphas, mus, kappas):
    """Host-side fp64 precompute of the tiny per-component constants."""
    a = np.asarray(alphas, np.float64)
    m = np.asarray(mus, np.float64)
    k = np.asarray(kappas, np.float64)
    d = m.shape[1]
    nu = 0.5 * d - 1.0
    z = k / nu
    sq = np.sqrt(1.0 + z * z)
    eta = sq + np.log(z) - np.log1p(sq)
    t = 1.0 / sq
    u1 = (3.0 * t - 5.0 * t ** 3) / 24.0
    u2 = (81.0 * t ** 2 - 462.0 * t ** 4 + 385.0 * t ** 6) / 1152.0
    log_iv = (nu * eta - 0.5 * np.log(2.0 * np.pi * nu)
              - 0.25 * np.log1p(z * z) + np.log1p(u1 / nu + u2 / (nu * nu)))
    logC = d * (-0.5 * np.log(2.0 * np.pi)) + nu * np.log(k) - log_iv
    coef = np.log(a) + np.log(k) + logC
    delta = (coef - coef.max()).astype(np.float32).reshape(K, 1)

    musk = k[:, None] * m                      # kappa_k * mus_k
    # muskT[p, c, j] = musk[j, 128c + p]
    muskT = np.ascontiguousarray(
        musk.reshape(K, 4, 128).transpose(2, 1, 0)).astype(np.float16)
    musr = np.asarray(mus, np.float64).astype(mybir.dt.np(BF16))
    nkh = (-1.0 / k)[:, None].astype(np.float32)
    redq = np.ones((K, 128), np.float16)
    return dict(muskT=muskT, delta=delta, musr=musr, nkh=nkh, redq=redq)


def pack_shard(shard16):
    """[PAD_ROWS, 512] fp16 -> [N_ST, 128, 4*ST_ROWS] packed transposed."""
    v = shard16.reshape(N_ST, ST_ROWS, 4, 128).transpose(0, 3, 2, 1)
    return np.ascontiguousarray(v).reshape(N_ST, 128, 4 * ST_ROWS)


_NC_CACHE = {}


def kernel(s, alphas, mus, kappas):
    global LAST_RESULT
    s = np.asarray(s, np.float32)
    consts = host_prep(alphas, mus, kappas)

    if PAD_ROWS not in _NC_CACHE:
        _NC_CACHE[PAD_ROWS] = build_nc(PAD_ROWS)
    nc = _NC_CACHE[PAD_ROWS]

    in_maps = []
    for c in range(N_CORES):
        shard = s[c * ROWS_PER_CORE:(c + 1) * ROWS_PER_CORE]
        pad = PAD_ROWS - shard.shape[0]
        if pad:
            shard = np.concatenate([shard, shard[:pad]], axis=0)
        in_maps.append({"sT": pack_shard(shard.astype(np.float16)), **consts})

    res = run_bass_kernel_spmd(
        nc, in_maps, list(range(N_CORES)),
        trace=bool(os.environ.get("MIXVMF_TRACE")),
    )
    LAST_RESULT = res

    outs = []
    for c in range(N_CORES):
        o = np.asarray(res.results[c]["o"])
        nq = np.asarray(res.results[c]["nq"], np.float32).reshape(PAD_ROWS)
        out = np.ascontiguousarray(
            o.view(np.float16).reshape(N_ST, 128, 4, ST_ROWS)
            .transpose(0, 3, 2, 1)).reshape(PAD_ROWS, D).astype(np.float32)
        q = -nq
        no2 = np.einsum("ij,ij->i", out, out)
        r = 1.0 / np.sqrt(no2 + q * q)
        out *= r[:, None]
        outs.append(out[:ROWS_PER_CORE])
    return np.concatenate(outs, axis=0)


# revision 20
# speedup vs baseline: 3.3941x; 1.0342x over previous
"""Trainium2 Bass kernel for nn_MixvMFGrad (mixture-of-vMF log-density gradient).

Math (per row s of the batch, d=512, K=64 components):
    dots  = s @ mus^T                        [K]
    t_k   = delta_k + kappa_k * dots_k       (delta = coef - max coef, host fp64)
    e     = exp(t)                           (unnormalized weights)
    g     = e @ mus                          [d]
    q     = g . s
    out   = (g - q s) / ||g||

Device computes o = g - q s (unnormalized) and q; the norm is recovered on
the host via Pythagoras: since ||s|| = 1, ||o||^2 = ||g||^2 - q^2, so
r = 1/sqrt(||o||^2 + q^2) and out = o * r. This removes the Ge matmul, the
e*Ge product, and the whole on-device rsqrt chain (ACT Rsqrt is banned and
exp/rsqrt live in different ACT table sets).

Layout: everything transposed ([d, rows] / [K, rows]), with s pre-transposed
and fp16-packed on the host so the device does ZERO transposes. I/O is fp16
both ways (51 MB/core total), sized against the ~150us/core DMA roofline.

The loop is SOFTWARE-PIPELINED one stage deep: iteration i issues
dots(i), negq(i-1), gT(i-1) on PE — every operand (e, u of stage i-1) was
produced during iteration i-1, so the nine N=512 matmuls run back-to-back
with no semaphore stalls. An unbroken PE stream matters twice: it removes
pipeline bubbles, and sustained execution is what lets the Tensor engine
ramp out of the mid pstate (measured 630ns/matmul at 1.2GHz effective vs
~390ns at full clock).

The q-reduce lands directly in broadcast form: negq_bc = redq^T @ u where
redq's 128 identical columns are 1 (the -1/kappa weights ride in A16's
per-partition scale), so every output partition holds -q[r] and the tangent
update needs no cross-partition broadcast. Engine split per supertile:
PE 9 matmuls; ACT 3 ops (nq16 drain, A16 drain, exp); Pool 1 op (u = e*A16;
SBUF-only, Pool has no PSUM port); DVE 2 muls (16-bit 2x mode) + 4 adds
(the only PSUM-sourced elementwise). PSUM: A 2 banks + negq 1 + gT 5 = 8.

Precision (numpy-emulated): rel err ~4e-3 vs fp64 truth (gate 2e-2),
measured 1.8e-3 on HW. fp16 ranges are safe: |A|<=25, e<=~250 (bf16),
|u|<=~1.5e3, |o|<=~40.
"""

import os
from contextlib import ExitStack

import numpy as np

import concourse.bass as bass
import concourse.tile as tile
from concourse import bacc
from concourse import mybir
from concourse.bass_utils import run_bass_kernel_spmd

N_CORES = 8
BS = 200000
D = 512
K = 64
ROWS_PER_CORE = BS // N_CORES   # 25000
ST_ROWS = 512                   # rows per supertile
PAD_ROWS = 25088                # 49 supertiles of 512
N_ST = PAD_ROWS // ST_ROWS
F32 = mybir.dt.float32
F16 = mybir.dt.float16
BF16 = mybir.dt.bfloat16

LAST_RESULT = None  # test.py reads exec_time_ns off this


def build_nc(rows=PAD_ROWS):
    assert rows % ST_ROWS == 0
    n_st = rows // ST_ROWS
    nc = bacc.Bacc("TRN2", target_bir_lowering=False)

    # packed layouts: x_d[st, p, c*R + r] = x[row = st*R + r, dim = c*128 + p]
    sT_d = nc.dram_tensor("sT", [n_st, 128, 4 * ST_ROWS], F16,
                          kind="ExternalInput")
    o_d = nc.dram_tensor("o", [n_st, 128, 4 * ST_ROWS], F16,
                         kind="ExternalOutput")
    nq_d = nc.dram_tensor("nq", [n_st, ST_ROWS], F16, kind="ExternalOutput")
    muskT_d = nc.dram_tensor("muskT", [128, 4, K], F16, kind="ExternalInput")
    delta_d = nc.dram_tensor("delta", [K, 1], F32, kind="ExternalInput")
    musr_d = nc.dram_tensor("musr", [K, D], BF16, kind="ExternalInput")
    nkh_d = nc.dram_tensor("nkh", [K, 1], F32, kind="ExternalInput")
    redq_d = nc.dram_tensor("redq", [K, 128], F16, kind="ExternalInput")

    AF = mybir.ActivationFunctionType

    sT_v = sT_d[:].rearrange("t p (c r) -> t p c r", r=ST_ROWS)
    o_v = o_d[:].rearrange("t p (c r) -> t p c r", r=ST_ROWS)
    nq_v = nq_d[:]

    with tile.TileContext(nc) as tc, ExitStack() as ctx:
        consts = ctx.enter_context(tc.tile_pool(name="consts", bufs=1))
        in_pool = ctx.enter_context(tc.tile_pool(name="in_pool", bufs=4))
        out_pool = ctx.enter_context(tc.tile_pool(name="out_pool", bufs=4))
        e_pool = ctx.enter_context(tc.tile_pool(name="e_pool", bufs=3))
        u_pool = ctx.enter_context(tc.tile_pool(name="u_pool", bufs=3))
        q_pool = ctx.enter_context(tc.tile_pool(name="q_pool", bufs=3))
        # PSUM budget (8 banks): A 1bank x 2bufs, negq 1 x 1, gT 1 x 5
        ps_A = ctx.enter_context(tc.tile_pool(name="ps_A", bufs=2, space="PSUM"))
        ps_Q = ctx.enter_context(tc.tile_pool(name="ps_Q", bufs=1, space="PSUM"))
        ps_G = ctx.enter_context(tc.tile_pool(name="ps_G", bufs=5, space="PSUM"))

        muskT_sb = consts.tile([128, 4, K], F16)
        nc.sync.dma_start(out=muskT_sb, in_=muskT_d[:])
        delta_sb = consts.tile([K, 1], F32)
        nc.sync.dma_start(out=delta_sb, in_=delta_d[:])
        musr_sb = consts.tile([K, D], BF16)
        nc.sync.dma_start(out=musr_sb, in_=musr_d[:])
        nkh_sb = consts.tile([K, 1], F32)
        nc.sync.dma_start(out=nkh_sb, in_=nkh_d[:])
        redq_sb = consts.tile([K, 128], F16)
        nc.sync.dma_start(out=redq_sb, in_=redq_d[:])

        prev = None  # state of stage i-1: dict(sT, o, e, u)
        for i in range(n_st + 1):
            cur = None
            if i < n_st:
                sT_t = in_pool.tile([128, 4, ST_ROWS], F16, tag="sT")
                nc.sync.dma_start(out=sT_t, in_=sT_v[i])
                o_t = out_pool.tile([128, 4, ST_ROWS], F16, tag="o")

                # A = (kappa*dots)^T [K, rows], fp32 PSUM
                A = ps_A.tile([K, ST_ROWS], F32, tag="A")
                for c in range(4):
                    nc.tensor.matmul(
                        A, muskT_sb[:, c, :], sT_t[:, c, :],
                        start=(c == 0), stop=(c == 3),
                    )
                cur = dict(sT=sT_t, o=o_t)

            if prev is not None:
                # -q(i-1) on all 128 partitions: redq cols are all ones,
                # u already carries the -1/kappa weights
                negq = ps_Q.tile([128, ST_ROWS], F32, tag="q")
                nc.tensor.matmul(negq, redq_sb, prev["u"], start=True,
                                 stop=True)
                nq16 = q_pool.tile([128, ST_ROWS], F16, tag="nq16")
                nc.scalar.copy(nq16, negq)
                nc.sync.dma_start(out=nq_v[i - 1:i], in_=nq16[0:1, :])

                # gT(i-1) per d-chunk
                gts = []
                for c in range(4):
                    gt = ps_G.tile([128, ST_ROWS], F32, tag="g")
                    nc.tensor.matmul(
                        gt, musr_sb[:, 128 * c:128 * (c + 1)], prev["e"],
                        start=True, stop=True)
                    gts.append(gt)

                # tmp = sT * (-q) (DVE 16-bit 2x, merged pairs, stride-0
                # broadcast of nq16), then o = tmp + gT per chunk
                po, ps = prev["o"], prev["sT"]
                nq_b = nq16[:].rearrange("p (o r) -> p o r", o=1).broadcast_to(
                    [128, 2, ST_ROWS])
                nc.vector.tensor_mul(po[:, 0:2, :], ps[:, 0:2, :], nq_b)
                nc.vector.tensor_mul(po[:, 2:4, :], ps[:, 2:4, :], nq_b)
                for c in range(4):
                    nc.vector.tensor_add(po[:, c, :], po[:, c, :], gts[c])
                # output store on the second hwdge queue (ACT) so the in/out
                # streams don't serialize dispatch on the sync queue
                nc.scalar.dma_start(out=o_v[i - 1], in_=po)

            if cur is not None:
                # ACT drains for stage i (after stage i-1's nq16 in the ACT
                # queue): A16 = A * (-1/kappa), e = exp(A + delta)
                A16 = e_pool.tile([K, ST_ROWS], F16, tag="A16")
                nc.scalar.mul(A16, A, nkh_sb)
                e_t = e_pool.tile([K, ST_ROWS], BF16, tag="e")
                nc.scalar.activation(e_t, A, AF.Exp, bias=delta_sb)
                # u = e * A16 (Pool engine: SBUF-only operands)
                u_t = u_pool.tile([K, ST_ROWS], F16, tag="u")
                nc.gpsimd.tensor_mul(u_t, e_t, A16)
                cur["e"] = e_t
                cur["u"] = u_t

            prev = cur

    nc.finalize()
    return nc


def host_prep(alphas, mus, kappas):
    """Host-side fp64 precompute of the tiny per-component constants."""
    a = np.asarray(alphas, np.float64)
    m = np.asarray(mus, np.float64)
    k = np.asarray(kappas, np.float64)
    d = m.shape[1]
    nu = 0.5 * d - 1.0
    z = k / nu
    sq = np.sqrt(1.0 + z * z)
    eta = sq + np.log(z) - np.log1p(sq)
    t = 1.0 / sq
    u1 = (3.0 * t - 5.0 * t ** 3) / 24.0
    u2 = (81.0 * t ** 2 - 462.0 * t ** 4 + 385.0 * t ** 6) / 1152.0
    log_iv = (nu * eta - 0.5 * np.log(2.0 * np.pi * nu)
              - 0.25 * np.log1p(z * z) + np.log1p(u1 / nu + u2 / (nu * nu)))
    logC = d * (-0.5 * np.log(2.0 * np.pi)) + nu * np.log(k) - log_iv
    coef = np.log(a) + np.log(k) + logC
    delta = (coef - coef.max()).astype(np.float32).reshape(K, 1)

    musk = k[:, None] * m                      # kappa_k * mus_k
    # muskT[p, c, j] = musk[j, 128c + p]
    muskT = np.ascontiguousarray(
        musk.reshape(K, 4, 128).transpose(2, 1, 0)).astype(np.float16)
    musr = np.asarray(mus, np.float64).astype(mybir.dt.np(BF16))
    nkh = (-1.0 / k)[:, None].astype(np.float32)
    redq = np.ones((K, 128), np.float16)
    return dict(muskT=muskT, delta=delta, musr=musr, nkh=nkh, redq=redq)


def pack_shard(shard16):
    """[PAD_ROWS, 512] fp16 -> [N_ST, 128, 4*ST_ROWS] packed transposed."""
    v = shard16.reshape(N_ST, ST_ROWS, 4, 128).transpose(0, 3, 2, 1)
    return np.ascontiguousarray(v).reshape(N_ST, 128, 4 * ST_ROWS)


_NC_CACHE = {}


def kernel(s, alphas, mus, kappas):
    global LAST_RESULT
    s = np.asarray(s, np.float32)
    consts = host_prep(alphas, mus, kappas)

    if PAD_ROWS not in _NC_CACHE:
        _NC_CACHE[PAD_ROWS] = build_nc(PAD_ROWS)
    nc = _NC_CACHE[PAD_ROWS]

    in_maps = []
    for c in range(N_CORES):
        shard = s[c * ROWS_PER_CORE:(c + 1) * ROWS_PER_CORE]
        pad = PAD_ROWS - shard.shape[0]
        if pad:
            shard = np.concatenate([shard, shard[:pad]], axis=0)
        in_maps.append({"sT": pack_shard(shard.astype(np.float16)), **consts})

    res = run_bass_kernel_spmd(
        nc, in_maps, list(range(N_CORES)),
        trace=bool(os.environ.get("MIXVMF_TRACE")),
    )
    LAST_RESULT = res

    outs = []
    for c in range(N_CORES):
        o = np.asarray(res.results[c]["o"])
        nq = np.asarray(res.results[c]["nq"], np.float32).reshape(PAD_ROWS)
        out = np.ascontiguousarray(
            o.view(np.float16).reshape(N_ST, 128, 4, ST_ROWS)
            .transpose(0, 3, 2, 1)).reshape(PAD_ROWS, D).astype(np.float32)
        q = -nq
        no2 = np.einsum("ij,ij->i", out, out)
        r = 1.0 / np.sqrt(no2 + q * q)
        out *= r[:, None]
        outs.append(out[:ROWS_PER_CORE])
    return np.concatenate(outs, axis=0)


# revision 21
# speedup vs baseline: 3.4122x; 1.0053x over previous
"""Trainium2 Bass kernel for nn_MixvMFGrad (mixture-of-vMF log-density gradient).

Math (per row s of the batch, d=512, K=64 components):
    dots  = s @ mus^T                        [K]
    t_k   = delta_k + kappa_k * dots_k       (delta = coef - max coef, host fp64)
    e     = exp(t)                           (unnormalized weights)
    g     = e @ mus                          [d]
    q     = g . s
    out   = (g - q s) / ||g||

Device computes o = g - q s (unnormalized) and q; the norm is recovered on
the host via Pythagoras: since ||s|| = 1, ||o||^2 = ||g||^2 - q^2, so
r = 1/sqrt(||o||^2 + q^2) and out = o * r. This removes the Ge matmul, the
e*Ge product, and the whole on-device rsqrt chain (ACT Rsqrt is banned and
exp/rsqrt live in different ACT table sets).

Layout: everything transposed ([d, rows] / [K, rows]), with s pre-transposed
and fp16-packed on the host so the device does ZERO transposes. I/O is fp16
both ways (51 MB/core total), sized against the ~150us/core DMA roofline.

The loop is SOFTWARE-PIPELINED one stage deep: iteration i issues
dots(i), negq(i-1), gT(i-1) on PE — every operand (e, u of stage i-1) was
produced during iteration i-1, so the nine N=512 matmuls run back-to-back
with no semaphore stalls. An unbroken PE stream matters twice: it removes
pipeline bubbles, and sustained execution is what lets the Tensor engine
ramp out of the mid pstate (measured 630ns/matmul at 1.2GHz effective vs
~390ns at full clock).

The q-reduce lands directly in broadcast form: negq_bc = redq^T @ u where
redq's 128 identical columns are 1 (the -1/kappa weights ride in A16's
per-partition scale), so every output partition holds -q[r] and the tangent
update needs no cross-partition broadcast. Engine split per supertile:
PE 9 matmuls; ACT 3 ops (nq16 drain, A16 drain, exp); Pool 1 op (u = e*A16;
SBUF-only, Pool has no PSUM port); DVE 2 muls (16-bit 2x mode) + 4 adds
(the only PSUM-sourced elementwise). PSUM: A 2 banks + negq 1 + gT 5 = 8.

Precision (numpy-emulated): rel err ~4e-3 vs fp64 truth (gate 2e-2),
measured 1.8e-3 on HW. fp16 ranges are safe: |A|<=25, e<=~250 (bf16),
|u|<=~1.5e3, |o|<=~40.
"""

import os
from contextlib import ExitStack

import numpy as np

import concourse.bass as bass
import concourse.tile as tile
from concourse import bacc
from concourse import mybir
from concourse.bass_utils import run_bass_kernel_spmd

N_CORES = 8
BS = 200000
D = 512
K = 64
ROWS_PER_CORE = BS // N_CORES   # 25000
ST_ROWS = 512                   # rows per supertile
PAD_ROWS = 25088                # 49 supertiles of 512
N_ST = PAD_ROWS // ST_ROWS
F32 = mybir.dt.float32
F16 = mybir.dt.float16
BF16 = mybir.dt.bfloat16

LAST_RESULT = None  # test.py reads exec_time_ns off this


def build_nc(rows=PAD_ROWS):
    assert rows % ST_ROWS == 0
    n_st = rows // ST_ROWS
    nc = bacc.Bacc("TRN2", target_bir_lowering=False)

    # packed layouts: x_d[st, p, c*R + r] = x[row = st*R + r, dim = c*128 + p]
    sT_d = nc.dram_tensor("sT", [n_st, 128, 4 * ST_ROWS], BF16,
                          kind="ExternalInput")
    o_d = nc.dram_tensor("o", [n_st, 128, 4 * ST_ROWS], F16,
                         kind="ExternalOutput")
    nq_d = nc.dram_tensor("nq", [n_st, ST_ROWS], BF16, kind="ExternalOutput")
    muskT_d = nc.dram_tensor("muskT", [128, 4, K], BF16, kind="ExternalInput")
    delta_d = nc.dram_tensor("delta", [K, 1], F32, kind="ExternalInput")
    musr_d = nc.dram_tensor("musr", [K, D], BF16, kind="ExternalInput")
    nkh_d = nc.dram_tensor("nkh", [K, 1], F32, kind="ExternalInput")
    redq_d = nc.dram_tensor("redq", [K, 128], BF16, kind="ExternalInput")

    AF = mybir.ActivationFunctionType

    sT_v = sT_d[:].rearrange("t p (c r) -> t p c r", r=ST_ROWS)
    o_v = o_d[:].rearrange("t p (c r) -> t p c r", r=ST_ROWS)
    nq_v = nq_d[:]

    with tile.TileContext(nc) as tc, ExitStack() as ctx:
        consts = ctx.enter_context(tc.tile_pool(name="consts", bufs=1))
        in_pool = ctx.enter_context(tc.tile_pool(name="in_pool", bufs=4))
        out_pool = ctx.enter_context(tc.tile_pool(name="out_pool", bufs=4))
        e_pool = ctx.enter_context(tc.tile_pool(name="e_pool", bufs=3))
        u_pool = ctx.enter_context(tc.tile_pool(name="u_pool", bufs=3))
        q_pool = ctx.enter_context(tc.tile_pool(name="q_pool", bufs=3))
        # PSUM budget (8 banks): A 1bank x 2bufs, negq 1 x 1, gT 1 x 5
        ps_A = ctx.enter_context(tc.tile_pool(name="ps_A", bufs=2, space="PSUM"))
        ps_Q = ctx.enter_context(tc.tile_pool(name="ps_Q", bufs=1, space="PSUM"))
        ps_G = ctx.enter_context(tc.tile_pool(name="ps_G", bufs=5, space="PSUM"))

        muskT_sb = consts.tile([128, 4, K], BF16)
        nc.sync.dma_start(out=muskT_sb, in_=muskT_d[:])
        delta_sb = consts.tile([K, 1], F32)
        nc.sync.dma_start(out=delta_sb, in_=delta_d[:])
        musr_sb = consts.tile([K, D], BF16)
        nc.sync.dma_start(out=musr_sb, in_=musr_d[:])
        nkh_sb = consts.tile([K, 1], F32)
        nc.sync.dma_start(out=nkh_sb, in_=nkh_d[:])
        redq_sb = consts.tile([K, 128], BF16)
        nc.sync.dma_start(out=redq_sb, in_=redq_d[:])

        prev = None  # state of stage i-1: dict(sT, o, e, u)
        for i in range(n_st + 1):
            cur = None
            if i < n_st:
                sT_t = in_pool.tile([128, 4, ST_ROWS], BF16, tag="sT")
                nc.sync.dma_start(out=sT_t, in_=sT_v[i])
                o_t = out_pool.tile([128, 4, ST_ROWS], F16, tag="o")

                # A = (kappa*dots)^T [K, rows], fp32 PSUM
                A = ps_A.tile([K, ST_ROWS], F32, tag="A")
                for c in range(4):
                    nc.tensor.matmul(
                        A, muskT_sb[:, c, :], sT_t[:, c, :],
                        start=(c == 0), stop=(c == 3),
                    )
                cur = dict(sT=sT_t, o=o_t)

            if prev is not None:
                # -q(i-1) on all 128 partitions: redq cols are all ones,
                # u already carries the -1/kappa weights
                negq = ps_Q.tile([128, ST_ROWS], F32, tag="q")
                nc.tensor.matmul(negq, redq_sb, prev["u"], start=True,
                                 stop=True)
                nq16 = q_pool.tile([128, ST_ROWS], BF16, tag="nq16")
                nc.scalar.copy(nq16, negq)
                nc.sync.dma_start(out=nq_v[i - 1:i], in_=nq16[0:1, :])

                # gT(i-1) per d-chunk
                gts = []
                for c in range(4):
                    gt = ps_G.tile([128, ST_ROWS], F32, tag="g")
                    nc.tensor.matmul(
                        gt, musr_sb[:, 128 * c:128 * (c + 1)], prev["e"],
                        start=True, stop=True)
                    gts.append(gt)

                # tmp = sT * (-q) (DVE 16-bit 2x, merged pairs, stride-0
                # broadcast of nq16), then o = tmp + gT per chunk
                po, ps = prev["o"], prev["sT"]
                nq_b = nq16[:].rearrange("p (o r) -> p o r", o=1).broadcast_to(
                    [128, 2, ST_ROWS])
                nc.vector.tensor_mul(po[:, 0:2, :], ps[:, 0:2, :], nq_b)
                nc.vector.tensor_mul(po[:, 2:4, :], ps[:, 2:4, :], nq_b)
                for c in range(4):
                    nc.vector.tensor_add(po[:, c, :], po[:, c, :], gts[c])
                # output store on the second hwdge queue (ACT) so the in/out
                # streams don't serialize dispatch on the sync queue
                nc.scalar.dma_start(out=o_v[i - 1], in_=po)

            if cur is not None:
                # ACT drains for stage i (after stage i-1's nq16 in the ACT
                # queue): A16 = A * (-1/kappa), e = exp(A + delta)
                A16 = e_pool.tile([K, ST_ROWS], BF16, tag="A16")
                nc.scalar.mul(A16, A, nkh_sb)
                e_t = e_pool.tile([K, ST_ROWS], BF16, tag="e")
                nc.scalar.activation(e_t, A, AF.Exp, bias=delta_sb)
                # u = e * A16 (Pool engine: SBUF-only operands)
                u_t = u_pool.tile([K, ST_ROWS], BF16, tag="u")
                nc.gpsimd.tensor_mul(u_t, e_t, A16)
                cur["e"] = e_t
                cur["u"] = u_t

            prev = cur

    nc.finalize()
    return nc


def host_prep(alphas, mus, kappas):
    """Host-side fp64 precompute of the tiny per-component constants."""
    a = np.asarray(alphas, np.float64)
    m = np.asarray(mus, np.float64)
    k = np.asarray(kappas, np.float64)
    d = m.shape[1]
    nu = 0.5 * d - 1.0
    z = k / nu
    sq = np.sqrt(1.0 + z * z)
    eta = sq + np.log(z) - np.log1p(sq)
    t = 1.0 / sq
    u1 = (3.0 * t - 5.0 * t ** 3) / 24.0
    u2 = (81.0 * t ** 2 - 462.0 * t ** 4 + 385.0 * t ** 6) / 1152.0
    log_iv = (nu * eta - 0.5 * np.log(2.0 * np.pi * nu)
              - 0.25 * np.log1p(z * z) + np.log1p(u1 / nu + u2 / (nu * nu)))
    logC = d * (-0.5 * np.log(2.0 * np.pi)) + nu * np.log(k) - log_iv
    coef = np.log(a) + np.log(k) + logC
    delta = (coef - coef.max()).astype(np.float32).reshape(K, 1)

    musk = k[:, None] * m                      # kappa_k * mus_k
    # muskT[p, c, j] = musk[j, 128c + p]
    muskT = np.ascontiguousarray(
        musk.reshape(K, 4, 128).transpose(2, 1, 0)).astype(mybir.dt.np(BF16))
    musr = np.asarray(mus, np.float64).astype(mybir.dt.np(BF16))
    nkh = (-1.0 / k)[:, None].astype(np.float32)
    redq = np.ones((K, 128), mybir.dt.np(BF16))
    return dict(muskT=muskT, delta=delta, musr=musr, nkh=nkh, redq=redq)


def pack_shard(shard16):
    """[PAD_ROWS, 512] fp16 -> [N_ST, 128, 4*ST_ROWS] packed transposed."""
    v = shard16.reshape(N_ST, ST_ROWS, 4, 128).transpose(0, 3, 2, 1)
    return np.ascontiguousarray(v).reshape(N_ST, 128, 4 * ST_ROWS)


_NC_CACHE = {}


def kernel(s, alphas, mus, kappas):
    global LAST_RESULT
    s = np.asarray(s, np.float32)
    consts = host_prep(alphas, mus, kappas)

    if PAD_ROWS not in _NC_CACHE:
        _NC_CACHE[PAD_ROWS] = build_nc(PAD_ROWS)
    nc = _NC_CACHE[PAD_ROWS]

    in_maps = []
    for c in range(N_CORES):
        shard = s[c * ROWS_PER_CORE:(c + 1) * ROWS_PER_CORE]
        pad = PAD_ROWS - shard.shape[0]
        if pad:
            shard = np.concatenate([shard, shard[:pad]], axis=0)
        in_maps.append({"sT": pack_shard(shard.astype(mybir.dt.np(BF16))), **consts})

    res = run_bass_kernel_spmd(
        nc, in_maps, list(range(N_CORES)),
        trace=bool(os.environ.get("MIXVMF_TRACE")),
    )
    LAST_RESULT = res

    outs = []
    for c in range(N_CORES):
        o = np.asarray(res.results[c]["o"])
        nq = np.asarray(res.results[c]["nq"], np.float32).reshape(PAD_ROWS)
        out = np.ascontiguousarray(
            o.view(np.float16).reshape(N_ST, 128, 4, ST_ROWS)
            .transpose(0, 3, 2, 1)).reshape(PAD_ROWS, D).astype(np.float32)
        q = -nq
        no2 = np.einsum("ij,ij->i", out, out)
        r = 1.0 / np.sqrt(no2 + q * q)
        out *= r[:, None]
        outs.append(out[:ROWS_PER_CORE])
    return np.concatenate(outs, axis=0)
